# revision 19
# baseline (speedup 1.0000x reference)
"""Trainium2 Bass kernel for nn_ControlFlexHNN (dense_mlp).

Data-parallel across 8 NeuronCores: batch N=32768 -> 4096 rows/core.
All activations are kept feature-major ("transposed": [feature, batch])
on-chip so every matmul contracts over the partition dimension.

Host-side work (part of sharding/layout, O(N*20) or O(H^2)):
  - u = tanh(z @ Wp.T + bp) (the detached policy, tiny)
  - zu = [z, u] transposed per-core shard
  - weight layout prep (transposes / chunking / bf16 cast)
  - final J-map: out = [s[:, DQ:], -s[:, :DQ]] and gather

Device kernel per core (B=512 batch tile, 8 tiles), math per tile:
  a1 = W1 @ zT;   h1 = tanh(a1+b1); s0 = 1-h1^2
  a2 = W2 @ h1;   s1 = 1-tanh(a2+b2)^2
  f1 = Wf1 @ zuT; g1 = tanh(f1+bf1) + (f1+bf1)*s1;  ga2 = s1*Wh
  gh = W2^T @ ga2;  ga1 = gh * s0
  f2 = Wf2 @ g1;  g2 = tanh(f2+bf2) + (f2+bf2)*s0
  sT = W1^T @ ga1 + Wff @ g2 + bff   (one PSUM accumulation)

Schedule: the three H-x-H GEMMs dominate PE time (64 matmuls each per
tile); everything else is hidden under them:
  - intermediates are bf16 in SBUF => DVE runs in 2x/4x perf modes
  - only the tanh ops run on ACT (2 per chunk instead of 4)
  - (psum+bias)*s fused into one scalar_tensor_tensor
  - tile t+1's layer-A matmuls+chains are embedded between tile t's BC
    and D loops (4 chunks, covering the ga2 chain) and in its EF loop,
    so the BC loop never waits on h1
  - head accums run 3 behind their g2 chains; the last three spill into
    tile t+1's BC loop so the chain latency is covered by real work
  - weights stream in per-128-column slices in consumption order so the
    first iteration starts after ~2 slices instead of the full 6.5MB

Matmuls: z-path (contract 16/20) in float32r; H-x-H and head paths in
bf16 (inputs rounded to bf16, fp32 PSUM accumulate).
"""

import numpy as np

N = 32768
DQ = 8
D2 = 2 * DQ          # 16
A_DIM = 4
ZU = D2 + A_DIM      # 20
H = 1024
HC = H // 128        # 8 chunks
NCORES = 8
NSH = N // NCORES    # 4096 rows per core
B = 512              # batch tile (free dim of matmuls)
TILES = NSH // B     # 8

_BUILT = None


def _build(loop_n=None):
    """Build the kernel. loop_n wraps the whole 8-tile body in an on-device
    For_i loop (used only for HW timing via replication differencing)."""
    import contextlib

    import concourse.bacc as bacc
    import concourse.mybir as mybir
    from concourse import tile

    f32 = mybir.dt.float32
    f32r = mybir.dt.float32r
    bf16 = mybir.dt.bfloat16

    nc = bacc.Bacc(None)

    zut_d = nc.dram_tensor("zut", [ZU, NSH], f32r, kind="ExternalInput")
    w1t_d = nc.dram_tensor("w1t", [D2, H], f32r, kind="ExternalInput")
    wf1t_d = nc.dram_tensor("wf1t", [ZU, H], f32r, kind="ExternalInput")
    # big weights, bf16, grouped so each 128-out-col slice is one
    # contiguous-per-partition DMA: [128p, slice, chunk, 128c]
    w2t_d = nc.dram_tensor("w2t", [128, HC, HC, 128], bf16, kind="ExternalInput")
    w2n_d = nc.dram_tensor("w2n", [128, HC, HC, 128], bf16, kind="ExternalInput")
    wf2t_d = nc.dram_tensor("wf2t", [128, HC, HC, 128], bf16, kind="ExternalInput")
    w1n_d = nc.dram_tensor("w1n", [128, HC, D2], bf16, kind="ExternalInput")
    wfft_d = nc.dram_tensor("wfft", [128, HC, D2], bf16, kind="ExternalInput")
    whc_d = nc.dram_tensor("whc", [128, HC], f32, kind="ExternalInput")
    b1c_d = nc.dram_tensor("b1c", [128, HC], f32, kind="ExternalInput")
    b2c_d = nc.dram_tensor("b2c", [128, HC], f32, kind="ExternalInput")
    bf1c_d = nc.dram_tensor("bf1c", [128, HC], f32, kind="ExternalInput")
    bf2c_d = nc.dram_tensor("bf2c", [128, HC], f32, kind="ExternalInput")
    bffc_d = nc.dram_tensor("bffc", [D2, 1], f32, kind="ExternalInput")
    st_d = nc.dram_tensor("st", [D2, NSH], f32, kind="ExternalOutput")

    with tile.TileContext(nc) as tc:
        with (
            tc.tile_pool(name="wp", bufs=1) as wp,
            tc.tile_pool(name="actp", bufs=2) as actp,
            tc.tile_pool(name="tmpp", bufs=4) as tmpp,
            tc.tile_pool(name="iop", bufs=3) as iop,
            tc.tile_pool(name="outp", bufs=2) as outp,
            tc.tile_pool(name="mmp", bufs=6, space="PSUM") as mmp,
            tc.tile_pool(name="accp", bufs=2, space="PSUM") as accp,
        ):
            # ---- resident weights, in consumption order; first two input
            # tiles ride ahead of the big weights so the prologue layer-A
            # matmuls start almost immediately ----
            w1t = wp.tile([D2, H], f32r)
            nc.sync.dma_start(w1t[:], w1t_d[:])
            zuts = [None] * TILES
            for t0 in (0, 1):
                zt = iop.tile([ZU, B], f32r, tag="zut01", name=f"zut_{t0}")
                nc.sync.dma_start(zt[:], zut_d[:, t0 * B:(t0 + 1) * B])
                zuts[t0] = zt
            b1c = wp.tile([128, HC], f32)
            nc.sync.dma_start(b1c[:], b1c_d[:])
            wf1t = wp.tile([ZU, H], f32r)
            nc.sync.dma_start(wf1t[:], wf1t_d[:])
            b2c = wp.tile([128, HC], f32)
            nc.sync.dma_start(b2c[:], b2c_d[:])
            bf1c = wp.tile([128, HC], f32)
            nc.sync.dma_start(bf1c[:], bf1c_d[:])
            bf2c = wp.tile([128, HC], f32)
            nc.sync.dma_start(bf2c[:], bf2c_d[:])
            whc = wp.tile([128, HC], f32)
            nc.sync.dma_start(whc[:], whc_d[:])
            bffc = wp.tile([D2, 1], f32)
            nc.sync.dma_start(bffc[:], bffc_d[:])
            # per-slice streams: consumer matmuls gate on single slices
            w2t = []
            for j in range(HC):
                wj = wp.tile([128, HC, 128], bf16, name=f"w2t_{j}")
                nc.sync.dma_start(wj[:], w2t_d[:, j])
                w2t.append(wj)
            w2n = []
            for k in range(HC):
                wk = wp.tile([128, HC, 128], bf16, name=f"w2n_{k}")
                nc.sync.dma_start(wk[:], w2n_d[:, k])
                w2n.append(wk)
            wf2t = []
            for j in range(HC):
                wj = wp.tile([128, HC, 128], bf16, name=f"wf2t_{j}")
                nc.sync.dma_start(wj[:], wf2t_d[:, j])
                wf2t.append(wj)
            w1n = wp.tile([128, HC, D2], bf16)
            nc.sync.dma_start(w1n[:], w1n_d[:])
            wfft = wp.tile([128, HC, D2], bf16)
            nc.sync.dma_start(wfft[:], wfft_d[:])

            weights = dict(w1t=w1t, wf1t=wf1t, w2t=w2t, w2n=w2n, wf2t=wf2t,
                           w1n=w1n, wfft=wfft, whc=whc, b1c=b1c, b2c=b2c,
                           bf1c=bf1c, bf2c=bf2c, bffc=bffc)
            pools = dict(actp=actp, tmpp=tmpp, iop=iop, outp=outp,
                         mmp=mmp, accp=accp, zuts=zuts)

            loop_cm = tc.For_i(0, loop_n, 1) if loop_n else contextlib.nullcontext()
            with loop_cm:
                _emit_body(nc, mybir, zut_d, st_d, weights, pools)

    nc.compile()
    return nc


def _build_looped(loop_n):
    return _build(loop_n=loop_n)


def _emit_body(nc, mybir, zut_d, st_d, W, P):
    f32 = mybir.dt.float32
    f32r = mybir.dt.float32r
    bf16 = mybir.dt.bfloat16
    Tanh = mybir.ActivationFunctionType.Tanh
    Square = mybir.ActivationFunctionType.Square
    mult = mybir.AluOpType.mult
    add = mybir.AluOpType.add

    actp, tmpp, iop, outp = P["actp"], P["tmpp"], P["iop"], P["outp"]
    mmp, accp = P["mmp"], P["accp"]

    zuts = P["zuts"]        # zut tiles; 0,1 pre-loaded before the weights
    h1s = [None] * TILES    # per-tile list of 8 h1 chunk tiles [128, B] bf16
    s0s = [None] * TILES    # per-tile s0 [128, HC, B] bf16

    def dma_zut(t):
        zt = iop.tile([ZU, B], f32r, tag="zut", name=f"zut_{t}")
        nc.sync.dma_start(zt[:], zut_d[:, t * B:(t + 1) * B])
        zuts[t] = zt

    def emit_A_chunk(t, j):
        """Layer A for tile t, chunk j: pa -> h1[t][:,j,:], s0[t][:,j,:]."""
        if j == 0:
            h1s[t] = []
            s0s[t] = actp.tile([128, HC, B], bf16, tag="s0", name=f"s0_{t}")
        h1c = actp.tile([128, B], bf16, tag="h1c", bufs=16, name=f"h1_{t}_{j}")
        h1s[t].append(h1c)
        pa = mmp.tile([128, B], f32, tag="mm", name=f"pa_{t}_{j}")
        nc.tensor.matmul(pa[:], W["w1t"][:, j * 128:(j + 1) * 128],
                         zuts[t][0:D2, :], start=True, stop=True)
        nc.scalar.activation(h1c[:], pa[:], Tanh, bias=W["b1c"][:, j:j + 1])
        hh = tmpp.tile([128, B], bf16, tag="hh", name=f"hh_{t}_{j}")
        nc.vector.tensor_tensor(out=hh[:], in0=h1c[:], in1=h1c[:], op=mult)
        nc.vector.tensor_scalar(out=s0s[t][:, j, :], in0=hh[:],
                                scalar1=-1.0, scalar2=1.0, op0=mult, op1=add)

    # ---------- prologue: layer A of tile 0 (zut 0,1 pre-loaded) ----------
    for j in range(HC):
        emit_A_chunk(0, j)

    prev = None  # (ps, g2buf, sout_tile, t) carried from tile t-1's EF loop

    for t in range(TILES):
        ga2 = [actp.tile([128, B], bf16, tag="ga2c", bufs=16, name=f"ga2_{t}_{j}")
               for j in range(HC)]
        g1 = [actp.tile([128, B], bf16, tag="g1c", bufs=16, name=f"g1_{t}_{j}")
              for j in range(HC)]

        # ---- BC loop: a2 -> s1; f1 -> g1; ga2 ----
        for j in range(HC):
            pb = mmp.tile([128, B], f32, tag="mm", name=f"pb_{t}_{j}")
            for k in range(HC):
                nc.tensor.matmul(pb[:], W["w2t"][j][:, k, :], h1s[t][k][:],
                                 start=(k == 0), stop=(k == HC - 1))
            pf = mmp.tile([128, B], f32, tag="mm", name=f"pf_{t}_{j}")
            nc.tensor.matmul(pf[:], W["wf1t"][:, j * 128:(j + 1) * 128],
                             zuts[t][:], start=True, stop=True)
            if prev is not None and j <= 2:
                # spill tile t-1's last three head accums here: their g2
                # chains finish under this tile's BC matmuls
                pps, pg2, _, pt = prev
                jj = HC - 3 + j
                nc.tensor.matmul(pps[:], W["wfft"][:, jj, :], pg2[:, jj, :],
                                 start=False, stop=(jj == HC - 1))
                if j == 2:
                    _, _, psout, pt = prev
                    nc.vector.tensor_scalar(out=psout[:], in0=pps[:],
                                            scalar1=W["bffc"][:, 0:1],
                                            scalar2=None, op0=add)
                    nc.sync.dma_start(st_d[:, pt * B:(pt + 1) * B], psout[:])
                    prev = None
            h2 = tmpp.tile([128, B], bf16, tag="h2", name=f"h2_{t}_{j}")
            nc.scalar.activation(h2[:], pb[:], Tanh, bias=W["b2c"][:, j:j + 1])
            hh2 = tmpp.tile([128, B], bf16, tag="hh", name=f"hh2_{t}_{j}")
            nc.vector.tensor_tensor(out=hh2[:], in0=h2[:], in1=h2[:], op=mult)
            s1 = tmpp.tile([128, B], bf16, tag="s1", name=f"s1_{t}_{j}")
            nc.vector.tensor_scalar(out=s1[:], in0=hh2[:], scalar1=-1.0,
                                    scalar2=1.0, op0=mult, op1=add)
            # ga2 first: the D loop's first accum group waits on all its
            # chunks, so ga2[7] is on the tile's critical path
            nc.vector.tensor_scalar(out=ga2[j][:], in0=s1[:],
                                    scalar1=W["whc"][:, j:j + 1],
                                    scalar2=None, op0=mult)
            th = tmpp.tile([128, B], bf16, tag="th", name=f"th_{t}_{j}")
            nc.scalar.activation(th[:], pf[:], Tanh, bias=W["bf1c"][:, j:j + 1])
            prod = tmpp.tile([128, B], bf16, tag="prod", name=f"prod_{t}_{j}")
            nc.vector.scalar_tensor_tensor(out=prod[:], in0=pf[:],
                                           scalar=W["bf1c"][:, j:j + 1],
                                           in1=s1[:], op0=add, op1=mult)
            nc.vector.tensor_tensor(out=g1[j][:], in0=th[:], in1=prod[:],
                                    op=add)

        if t + 2 < TILES:
            dma_zut(t + 2)
        if t + 1 < TILES:
            # independent matmuls of cover while ga2[7]'s chain drains
            # (the D loop's first accum group waits on the whole ga2 tile)
            for j in range(4):
                emit_A_chunk(t + 1, j)

        # ---- D loop: gh -> ga1 -> dH accum (pipelined 1 behind) ----
        ps = accp.tile([D2, B], f32, tag="acc", name=f"ps_{t}")
        ga1 = actp.tile([128, HC, B], bf16, tag="ga1", name=f"ga1_{t}")
        for k in range(HC):
            pg = mmp.tile([128, B], f32, tag="mm", name=f"pg_{t}_{k}")
            for j in range(HC):
                nc.tensor.matmul(pg[:], W["w2n"][k][:, j, :], ga2[j][:],
                                 start=(j == 0), stop=(j == HC - 1))
            nc.vector.tensor_tensor(out=ga1[:, k, :], in0=pg[:],
                                    in1=s0s[t][:, k, :], op=mult)
            if k >= 1:
                nc.tensor.matmul(ps[:], W["w1n"][:, k - 1, :],
                                 ga1[:, k - 1, :], start=(k == 1), stop=False)

        # ---- EF loop: f2 -> g2 -> head accum (pipelined 2 behind);
        #      embeds tile t+1's layer A ----
        g2buf = actp.tile([128, HC, B], bf16, tag="g2", name=f"g2_{t}")
        sout = outp.tile([D2, B], f32, tag="sout", name=f"sout_{t}")
        for j in range(HC):
            pf2 = mmp.tile([128, B], f32, tag="mm", name=f"pf2_{t}_{j}")
            for k in range(HC):
                nc.tensor.matmul(pf2[:], W["wf2t"][j][:, k, :], g1[k][:],
                                 start=(k == 0), stop=(k == HC - 1))
            if j == 1:
                # dH tail: ga1[7]'s chain finishes under pf2[0..1]'s matmuls
                nc.tensor.matmul(ps[:], W["w1n"][:, HC - 1, :],
                                 ga1[:, HC - 1, :], start=False, stop=False)
            if t + 1 < TILES and j >= 4:
                emit_A_chunk(t + 1, j)
            th2 = tmpp.tile([128, B], bf16, tag="th2", name=f"th2_{t}_{j}")
            nc.scalar.activation(th2[:], pf2[:], Tanh,
                                 bias=W["bf2c"][:, j:j + 1])
            prod2 = tmpp.tile([128, B], bf16, tag="prod", name=f"prod2_{t}_{j}")
            nc.vector.scalar_tensor_tensor(out=prod2[:], in0=pf2[:],
                                           scalar=W["bf2c"][:, j:j + 1],
                                           in1=s0s[t][:, j, :],
                                           op0=add, op1=mult)
            nc.vector.tensor_tensor(out=g2buf[:, j, :], in0=th2[:],
                                    in1=prod2[:], op=add)
            if j >= 3:
                nc.tensor.matmul(ps[:], W["wfft"][:, j - 3, :],
                                 g2buf[:, j - 3, :], start=False, stop=False)

        prev = (ps, g2buf, sout, t)

    # ---- drain: last tile's head accum tail + store ----
    ps, g2buf, sout, t = prev
    for jj in (HC - 3, HC - 2, HC - 1):
        nc.tensor.matmul(ps[:], W["wfft"][:, jj, :], g2buf[:, jj, :],
                         start=False, stop=(jj == HC - 1))
    nc.vector.tensor_scalar(out=sout[:], in0=ps[:], scalar1=W["bffc"][:, 0:1],
                            scalar2=None, op0=add)
    nc.sync.dma_start(st_d[:, t * B:(t + 1) * B], sout[:])


def _prep_inputs(t, z, W1, b1, W2, b2, Wh, bh, Wf1, bf1, Wf2, bf2, Wff, bff,
                 Wp, bp):
    import ml_dtypes

    f = np.float32
    b16 = ml_dtypes.bfloat16
    z = np.asarray(z, f)
    u = np.tanh(z @ np.asarray(Wp, f).T + np.asarray(bp, f))
    zu = np.concatenate([z, u], axis=1)          # [N, 20]

    def c(x, dt=f):
        return np.ascontiguousarray(np.asarray(x, f).astype(dt))

    W2 = np.asarray(W2, f)
    Wf2 = np.asarray(Wf2, f)
    W1 = np.asarray(W1, f)
    Wff = np.asarray(Wff, f)

    # [128p, out-slice j, contract-chunk k, 128c]
    def grp_t(M):  # lhsT for out = M @ x  (M: [H, H]); slice j = out cols
        # lhsT[p, j, k, c] = M[j*128+c, k*128+p]
        return np.ascontiguousarray(
            M.reshape(HC, 128, HC, 128).transpose(3, 0, 2, 1).astype(b16))

    def grp_n(M):  # lhsT for out = M^T @ x; slice k = out cols
        # lhsT[p, k, j, c] = M[j*128+p, k*128+c]
        return np.ascontiguousarray(
            M.reshape(HC, 128, HC, 128).transpose(1, 2, 0, 3).astype(b16))

    shared = {
        "w1t": c(W1.T),                                    # [16, 1024] f32r
        "wf1t": c(np.asarray(Wf1, f).T),                   # [20, 1024] f32r
        "w2t": grp_t(W2),
        "w2n": grp_n(W2),
        "wf2t": grp_t(Wf2),
        # w1n[p, k, m] = W1[k*128+p, m]
        "w1n": np.ascontiguousarray(
            W1.reshape(HC, 128, D2).transpose(1, 0, 2).astype(b16)),
        # wfft[p, j, m] = Wff[m, j*128+p]
        "wfft": np.ascontiguousarray(
            Wff.T.reshape(HC, 128, D2).transpose(1, 0, 2).astype(b16)),
        "whc": c(np.asarray(Wh, f).reshape(HC, 128).T),
        "b1c": c(np.asarray(b1, f).reshape(HC, 128).T),
        "b2c": c(np.asarray(b2, f).reshape(HC, 128).T),
        "bf1c": c(np.asarray(bf1, f).reshape(HC, 128).T),
        "bf2c": c(np.asarray(bf2, f).reshape(HC, 128).T),
        "bffc": c(np.asarray(bff, f).reshape(D2, 1)),
    }
    in_maps = []
    for r in range(NCORES):
        m = dict(shared)
        m["zut"] = np.ascontiguousarray(zu[r * NSH:(r + 1) * NSH].T.astype(f))
        in_maps.append(m)
    return in_maps


def _postprocess(results):
    outs = []
    for r in range(NCORES):
        s = results[r]["st"].T                    # [NSH, 16]
        outs.append(np.concatenate([s[:, DQ:], -s[:, :DQ]], axis=1))
    return np.ascontiguousarray(np.concatenate(outs, axis=0).astype(np.float32))


def kernel(**inputs):
    global _BUILT
    from concourse.bass_utils import run_bass_kernel_spmd

    if _BUILT is None:
        _BUILT = _build()
    in_maps = _prep_inputs(**inputs)
    res = run_bass_kernel_spmd(_BUILT, in_maps, list(range(NCORES)))
    return _postprocess(res.results)
